# revision 38
# baseline (speedup 1.0000x reference)
"""Trainium2 Bass kernel for nn_AC_Filter_PreNorm_Net (causal MHA, embed_dim=3,
L=2048, B=32) + post-attention integrator chain, data-parallel over 8 cores.

Algebraic reduction (see _host_prep): everything after the softmax collapses
into out^T[8, q] = (M @ [N; D])[., q] / D[q] where N/D are the unnormalized
softmax numerator rows (8) and denominator, accumulated by per-key-tile
matmuls with lhsT vm[k, 9].

v3 = the proven v1 schedule (full-mode 65-row scores keep the PE's HAM
activity monitor un-throttled at 2.4 GHz; 2-bank score tiles; staircase-
packed diagonal; gap-free exps) plus two changes:

  exp split:  alternating off-diagonal score groups use a VectorE 1-op
              fast-exp (i16 = round(s*128*log2e + magic), bitcast bf16 =
              2^frac mantissa-linear approx, +-3.3% max on ~35% of the
              off-diagonal weight mass) instead of ScalarE's spline exp,
              cutting the exp-pipeline bottleneck ~1.5x.
  PV tiling:  the numerator/denominator accumulation runs as col-tiled
              concurrent matmuls (M=32 strips at PSUM partitions 0/32/64/96,
              strip = kt mod 4), cutting TensorE's PV time ~3x. The four
              partial strips are summed on the host during unshard.
"""

import os
import sys
import math

import numpy as np
import ml_dtypes

BF16_NP = ml_dtypes.bfloat16

for _p in ("/opt/trn_rl_repo",):
    if os.path.isdir(_p) and _p not in sys.path:
        sys.path.append(_p)

import concourse.bacc as bacc
import concourse.tile as tile
from concourse import mybir
from concourse.bass_utils import run_bass_kernel_spmd

B, L, D = 32, 2048, 3
NCORES = 8
BPC = B // NCORES          # batches per core
QCH = 512                  # q-chunk width (one fp32 PSUM bank)
NQC = L // QCH
KTILE = 128                # keys per tile (partition dim)
NKT = L // KTILE
DT = 0.01
EPS = 1e-5
F32 = mybir.dt.float32
BF16 = mybir.dt.bfloat16
I16 = mybir.dt.int16

# fast-exp constants: i16 = round(s * C1 + C2); bitcast(i16) as bf16 ~ e^s
FE_C1 = 128.0 * 1.4426950408889634
FE_C2 = 127.0 * 128.0 - 5.51

_built = None              # cached compiled Bass graph

# exec_time_ns of the last traced run (None unless BASS_KERNEL_TRACE=1)
LAST_EXEC_TIME_NS = None


def _build(num_devices=NCORES):
    from contextlib import ExitStack

    nc = bacc.Bacc("TRN2", target_bir_lowering=False, debug=False,
                   num_devices=num_devices)

    q_d = nc.dram_tensor("q", [BPC, 3, L], BF16, kind="ExternalInput").ap()
    k_d = nc.dram_tensor("k", [BPC, 3, L], BF16, kind="ExternalInput").ap()
    vm_d = nc.dram_tensor("vm", [BPC, 128, (NKT + 1) * 32], BF16,
                          kind="ExternalInput").ap()
    mk_d = nc.dram_tensor("mask", [128, 256], BF16, kind="ExternalInput").ap()
    y_d = nc.dram_tensor("y", [BPC, NQC, 4, 9, QCH], F32,
                         kind="ExternalOutput").ap()

    with tile.TileContext(nc) as tc, ExitStack() as ctx:
        singles = ctx.enter_context(tc.tile_pool(name="singles", bufs=1))
        io_pool = ctx.enter_context(tc.tile_pool(name="io", bufs=2))
        e_pool = ctx.enter_context(tc.tile_pool(name="e", bufs=4))
        out_pool = ctx.enter_context(tc.tile_pool(name="out", bufs=2))
        s_pool = ctx.enter_context(tc.tile_pool(name="s", bufs=2, space="PSUM"))
        acc_pool = ctx.enter_context(
            tc.tile_pool(name="acc", bufs=2, space="PSUM"))

        mask_sb = singles.tile([128, 256], BF16)
        nc.sync.dma_start(out=mask_sb[:], in_=mk_d[:])

        # dummy activation with no deps: pulls the ~2.7us exp-table load
        # to kernel start, overlapping the input DMAs
        warm = singles.tile([1, 8], F32)
        nc.vector.memset(warm[:], 0.0)
        nc.scalar.activation(warm[:], warm[:],
                             mybir.ActivationFunctionType.Exp)

        # PE warmup burst: ~4us of full-mode matmuls un-throttles the HAM
        # clock gate before the real stream begins
        warm_w = singles.tile([128, 512], BF16)
        nc.vector.memset(warm_w[:], 0.0)
        warm_ps = acc_pool.tile([128, QCH], F32, tag="acc")
        for _ in range(10):
            nc.tensor.matmul(warm_ps[:], lhsT=warm_w[:, 0:128],
                             rhs=warm_w[:], start=True, stop=True)

        # persistent double-buffered q/k tiles: rows 0-2 hold the per-batch
        # projections (tiny DMAs), rows 3-127 zeroed once so the score
        # matmuls run a full-mode K=128 zero-padded contraction
        qk_sets = []
        for pi in range(2):
            q_sb = singles.tile([128, L], BF16, name=f"qsb{pi}")
            k_sb = singles.tile([128, L], BF16, name=f"ksb{pi}")
            nc.gpsimd.memset(q_sb[:], 0.0)
            nc.gpsimd.memset(k_sb[:], 0.0)
            qk_sets.append((q_sb, k_sb))

        # batches 0-1 run their q-chunks sequentially; batches 2-3 are
        # INTERLEAVED (big chunks paired with small) so the kernel tail
        # drains two independent dependency chains in parallel instead of
        # serializing one batch's exp->PV->copy chain
        units = [(b, qc) for b in (0, 1) for qc in range(NQC)]
        units += [(2, 0), (3, 3), (2, 1), (3, 2), (2, 2), (3, 1), (2, 3),
                  (3, 0)]
        vm_tiles = {}
        for b, qc in units:
            if b not in vm_tiles:
                q_sb, k_sb = qk_sets[b % 2]
                nc.sync.dma_start(out=q_sb[0:3, :], in_=q_d[b])
                nc.sync.dma_start(out=k_sb[0:3, :], in_=k_d[b])
                vm_sb = io_pool.tile([128, (NKT + 1) * 32], BF16, tag="vm")
                nc.sync.dma_start(out=vm_sb[:, 0:8 * 32],
                                  in_=vm_d[b][:, 0:8 * 32])
                nc.sync.dma_start(out=vm_sb[:, 8 * 32:(NKT + 1) * 32],
                                  in_=vm_d[b][:, 8 * 32:(NKT + 1) * 32])
                vm_tiles[b] = vm_sb
            q_sb, k_sb = qk_sets[b % 2]
            vm_sb = vm_tiles[b]
            if True:
                acc = acc_pool.tile([128, QCH], F32, tag="acc")
                strip_started = [False] * 4
                if b == 0:
                    # full-mode +0 accumulation: invisible to the output but
                    # HAM-visible filler that keeps the PE clock un-throttled
                    # through the ramp (the tiled PV matmuls don't count as
                    # PE activity for the HAM)
                    nc.tensor.matmul(acc[:], lhsT=warm_w[:, 0:128],
                                     rhs=warm_w[:], start=True, stop=False,
                                     skip_group_check=True)
                    strip_started = [True] * 4
                n_kt = 4 * qc + 4
                d0 = 4 * qc
                # groups of (kt, s-offset, width, acc-col-offset) sharing one
                # 3-bank PSUM tile and ONE gap-free exp each. Non-diagonal
                # tiles in triples; the 4 diagonal staircase tiles pack as
                # [d0 | d1 d3 | d2] -> 1280 contiguous cols, one exp.
                kts = list(range(4 * qc))
                groups = [[(kt, (j % 3) * QCH, QCH, 0)
                           for j, kt in enumerate(kts[i:i + 3])]
                          for i in range(0, len(kts), 3)]
                groups.append([(d0 + 0, 0, 512, 0),
                               (d0 + 1, 512, 384, 128),
                               (d0 + 2, 1024, 256, 256),
                               (d0 + 3, 896, 128, 384)])
                for gi, group in enumerate(groups):
                    is_diag = gi == len(groups) - 1
                    use_dve = (not is_diag) and (gi % 2 == 1)
                    if b == 0 and gi > 0 and gi % 2 == 0:
                        nc.tensor.matmul(acc[:], lhsT=warm_w[:, 0:128],
                                         rhs=warm_w[:], start=False,
                                         stop=False, skip_group_check=True)
                    s = s_pool.tile([128, 1536], F32)
                    for kt, soff, w, co in group:
                        nc.tensor.matmul(
                            s[:, soff:soff + w],
                            lhsT=k_sb[:, kt * KTILE:(kt + 1) * KTILE],
                            rhs=q_sb[:, qc * QCH + co:(qc + 1) * QCH],
                            start=True, stop=True)
                    e = e_pool.tile([128, 1536], BF16)
                    hi = max(soff + w for _, soff, w, _ in group)
                    if use_dve:
                        nc.vector.tensor_scalar(
                            e[:, 0:hi].bitcast(I16), s[:, 0:hi],
                            FE_C1, FE_C2,
                            mybir.AluOpType.mult, mybir.AluOpType.add)
                    else:
                        nc.scalar.activation(
                            e[:, 0:hi], s[:, 0:hi],
                            mybir.ActivationFunctionType.Exp)
                    if is_diag:
                        for eoff, mw in ((0, 128), (512, 128), (896, 256)):
                            nc.vector.tensor_mul(
                                e[:, eoff:eoff + mw], e[:, eoff:eoff + mw],
                                mask_sb[:, 0:mw])
                    if qc == 0 and gi == 0:
                        # zero-weight fills for the staircase's never-written
                        # column ranges so the acc bank is fully initialized
                        for ps in range(1, 4):
                            nc.tensor.matmul(
                                acc[32 * ps:32 * ps + 32, 0:128 * ps],
                                lhsT=vm_sb[:, NKT * 32:(NKT + 1) * 32],
                                rhs=e[:, 0:128 * ps],
                                start=not strip_started[ps], stop=False,
                                tile_position=(0, 32 * ps),
                                skip_group_check=True)
                            strip_started[ps] = True
                    for kt, soff, w, co in group:
                        ps = kt % 4
                        nc.tensor.matmul(
                            acc[32 * ps:32 * ps + 32, co:co + w],
                            lhsT=vm_sb[:, kt * 32:(kt + 1) * 32],
                            rhs=e[:, soff:soff + w],
                            start=not strip_started[ps],
                            stop=(kt >= d0),
                            tile_position=(0, 32 * ps),
                            skip_group_check=True)
                        strip_started[ps] = True

                out_sb = out_pool.tile([128, QCH], F32)
                nc.vector.tensor_copy(out_sb[:], acc[:])
                # per-chunk output DMAs on the (idle) gpsimd queue so they
                # never block the next batch's input DMAs on the sync queue;
                # the last batch's outputs use the lower-latency HWDGE sync
                # queue to shorten the tail
                out_eng = nc.sync if b == BPC - 1 else nc.gpsimd
                for ps in range(4):
                    out_eng.dma_start(
                        out=y_d[b, qc, ps],
                        in_=out_sb[32 * ps:32 * ps + 9, :])

    nc.compile()
    return nc


def _host_prep(inputs):
    """Fold the network's parameters into q/k projections and the VM matrix,
    and build per-core device inputs."""
    x = np.asarray(inputs["inputs"], dtype=np.float32)          # [B, L, 3]
    Wi = np.asarray(inputs["in_proj_w"], dtype=np.float64)      # [9, 3]
    bi = np.asarray(inputs["in_proj_b"], dtype=np.float64)      # [9]
    Wo = np.asarray(inputs["out_proj_w"], dtype=np.float64)     # [3, 3]
    bo = np.asarray(inputs["out_proj_b"], dtype=np.float64)     # [3]
    sigma = np.asarray(inputs["sigma"], dtype=np.float64)       # [2]
    f1_w = np.asarray(inputs["f1_w"], dtype=np.float64)
    f1_b = np.asarray(inputs["f1_b"], dtype=np.float64)
    f2_w = np.asarray(inputs["f2_w"], dtype=np.float64)
    f2_b = np.asarray(inputs["f2_b"], dtype=np.float64)
    g1_w = np.asarray(inputs["g1_w"], dtype=np.float64)
    g1_b = np.asarray(inputs["g1_b"], dtype=np.float64)
    g2_w = np.asarray(inputs["g2_w"], dtype=np.float64)
    g2_b = np.asarray(inputs["g2_b"], dtype=np.float64)
    m1 = float(np.asarray(inputs["m1_s"]))
    m2 = float(np.asarray(inputs["m2_s"]))

    scale = sigma + EPS
    dvec = np.array([1.0, 1.0 / scale[0], 1.0 / scale[1]])
    s3 = math.sqrt(3.0)

    Wq, Wk, Wv = Wi[0:3], Wi[3:6], Wi[6:9]
    bq, bk, bv = bi[0:3], bi[3:6], bi[6:9]
    Wq_eff = (Wq * dvec[None, :]) / s3
    bq_eff = bq / s3
    Wk_eff = Wk * dvec[None, :]
    bk_eff = bk
    Wv_eff = Wv * dvec[None, :]
    bv_eff = bv

    # affine collapse of the post-attention network: states are affine in
    # u = [1, a1, a2] (a = attention output channels 1, 2)
    e1 = np.array([1.0, 0.0, 0.0])

    def G(P):
        r1 = m1 * (g1_w @ P + g1_b[:, None] * e1[None, :])
        r2 = m2 * (g2_w @ P + g2_b[:, None] * e1[None, :])
        return np.vstack([np.zeros((1, 3)), r1, r2])

    P1 = np.eye(3)
    P2 = P1 + DT * G(P1)
    P3 = P2 + DT * G(P2)
    P4 = P3 + DT * G(P3)
    r7 = P4[1, :] + DT * m1 * (f1_w @ P4 + f1_b[:, None] * e1[None, :])[0]
    r8 = P4[2, :] + DT * m2 * (f2_w @ P4 + f2_b[:, None] * e1[None, :])[0]
    A = np.vstack([
        scale[0] * P2[1, :], scale[1] * P2[2, :],
        scale[0] * P3[1, :], scale[1] * P3[2, :],
        scale[0] * P4[1, :], scale[1] * P4[2, :],
        scale[0] * r7, scale[1] * r8,
    ])                                                  # [8, 3] in u-space
    U = np.zeros((3, 4))                                # u = U @ [ctx; 1]
    U[0, 3] = 1.0
    U[1, 0:3] = Wo[1, :]
    U[1, 3] = bo[1]
    U[2, 0:3] = Wo[2, :]
    U[2, 3] = bo[2]
    M = A @ U                                           # [8, 4]

    # vm: per-key row [ (V_ext @ M^T)[k], 1 ]  with V_ext = [V | 1]
    WvT_ext = np.zeros((4, 4))
    WvT_ext[0:3, 0:3] = Wv_eff.T
    WvT_ext[3, 0:3] = bv_eff
    WvT_ext[3, 3] = 1.0
    WVM = np.zeros((4, 9))
    WVM[:, 0:8] = WvT_ext @ M.T
    WVM[3, 8] = 1.0                     # softmax denominator column

    x_aug = np.concatenate([x, np.ones((B, L, 1), np.float32)], axis=-1)
    Wq_augT = np.concatenate([Wq_eff.T, bq_eff[None, :]],
                             axis=0).astype(np.float32)          # [4, 3]
    Wk_augT = np.concatenate([Wk_eff.T, bk_eff[None, :]],
                             axis=0).astype(np.float32)
    q_t = np.einsum("bld,dc->bcl", x_aug, Wq_augT)               # [B, 3, L]
    k_t = np.einsum("bld,dc->bcl", x_aug, Wk_augT)
    vm = x_aug @ WVM.astype(np.float32)                          # [B, L, 9]

    q_host = q_t.astype(BF16_NP)                                 # [B, 3, L]
    k_host = k_t.astype(BF16_NP)

    # vm per key tile, padded to 32 cols (9 real + zeros) + one zero slot
    vm_pad = np.zeros((B, NKT + 1, KTILE, 32), dtype=BF16_NP)
    vm_pad[:, 0:NKT, :, 0:9] = vm.reshape(B, NKT, KTILE, 9).astype(BF16_NP)
    vm_dev = np.ascontiguousarray(
        vm_pad.transpose(0, 2, 1, 3).reshape(B, KTILE, (NKT + 1) * 32))

    tri = (np.arange(128)[None, :] >=
           np.arange(128)[:, None]).astype(BF16_NP)
    mask = np.concatenate([tri, tri], axis=1)                    # [128, 256]
    in_maps = []
    for c in range(NCORES):
        sl = slice(c * BPC, (c + 1) * BPC)
        in_maps.append({
            "q": np.ascontiguousarray(q_host[sl]),
            "k": np.ascontiguousarray(k_host[sl]),
            "vm": np.ascontiguousarray(vm_dev[sl]),
            "mask": mask,
        })
    return in_maps


def _unshard(y):
    """[B, NQC, 4, 9, QCH] strip partials -> [B, L, 8] output."""
    # zero the never-written acc regions of qc=0 (strip s valid from col 128s)
    for s in range(1, 4):
        y[:, 0, s, :, 0:128 * s] = 0.0
    acc = y.sum(axis=2)                               # [B, NQC, 9, QCH]
    nb = y.shape[0]
    acc = acc.transpose(0, 2, 1, 3).reshape(nb, 9, L)
    num = acc[:, 0:8, :]
    den = acc[:, 8:9, :]
    out = (num / den).transpose(0, 2, 1)              # [B, L, 8]
    return np.ascontiguousarray(out.astype(np.float32))


def kernel(**inputs) -> np.ndarray:
    global _built, LAST_EXEC_TIME_NS
    if _built is None:
        _built = _build()
    nc = _built

    in_maps = _host_prep(inputs)

    trace = os.environ.get("BASS_KERNEL_TRACE", "") == "1"
    res = run_bass_kernel_spmd(nc, in_maps, list(range(NCORES)), trace=trace)
    if trace:
        LAST_EXEC_TIME_NS = res.exec_time_ns

    y = np.concatenate([res.results[c]["y"] for c in range(NCORES)],
                       axis=0)                        # [B, NQC, 4, 9, QCH]
    return _unshard(y)


# revision 40
# speedup vs baseline: 1.1383x; 1.1383x over previous
"""Trainium2 Bass kernel for nn_AC_Filter_PreNorm_Net (causal MHA, embed_dim=3,
L=2048, B=32) + post-attention integrator chain, data-parallel over 8 cores.

Algebraic reduction (see _host_prep): everything after the softmax collapses
into out^T[8, q] = (M @ [N; D])[., q] / D[q] where N/D are the unnormalized
softmax numerator rows (8) and denominator, accumulated by per-key-tile
matmuls with lhsT vm[k, 9].

v3 = the proven v1 schedule (full-mode 65-row scores keep the PE's HAM
activity monitor un-throttled at 2.4 GHz; 2-bank score tiles; staircase-
packed diagonal; gap-free exps) plus two changes:

  exp split:  alternating off-diagonal score groups use a VectorE 1-op
              fast-exp (i16 = round(s*128*log2e + magic), bitcast bf16 =
              2^frac mantissa-linear approx, +-3.3% max on ~35% of the
              off-diagonal weight mass) instead of ScalarE's spline exp,
              cutting the exp-pipeline bottleneck ~1.5x.
  PV tiling:  the numerator/denominator accumulation runs as col-tiled
              concurrent matmuls (M=32 strips at PSUM partitions 0/32/64/96,
              strip = kt mod 4), cutting TensorE's PV time ~3x. The four
              partial strips are summed on the host during unshard.
"""

import os
import sys
import math

import numpy as np
import ml_dtypes

BF16_NP = ml_dtypes.bfloat16

for _p in ("/opt/trn_rl_repo",):
    if os.path.isdir(_p) and _p not in sys.path:
        sys.path.append(_p)

import concourse.bacc as bacc
import concourse.tile as tile
from concourse import mybir
from concourse.bass_utils import run_bass_kernel_spmd

B, L, D = 32, 2048, 3
NCORES = 8
BPC = B // NCORES          # batches per core
QCH = 512                  # q-chunk width (one fp32 PSUM bank)
NQC = L // QCH
KTILE = 128                # keys per tile (partition dim)
NKT = L // KTILE
DT = 0.01
EPS = 1e-5
F32 = mybir.dt.float32
BF16 = mybir.dt.bfloat16
I16 = mybir.dt.int16

# fast-exp constants: i16 = round(s * C1 + C2); bitcast(i16) as bf16 ~ e^s
FE_C1 = 128.0 * 1.4426950408889634
FE_C2 = 127.0 * 128.0 - 5.51

_built = None              # cached compiled Bass graph

# exec_time_ns of the last traced run (None unless BASS_KERNEL_TRACE=1)
LAST_EXEC_TIME_NS = None


def _build(num_devices=NCORES):
    from contextlib import ExitStack

    nc = bacc.Bacc("TRN2", target_bir_lowering=False, debug=False,
                   num_devices=num_devices)

    q_d = nc.dram_tensor("q", [BPC, 3, L], BF16, kind="ExternalInput").ap()
    k_d = nc.dram_tensor("k", [BPC, 3, L], BF16, kind="ExternalInput").ap()
    vm_d = nc.dram_tensor("vm", [BPC, 128, (NKT + 1) * 32], BF16,
                          kind="ExternalInput").ap()
    mk_d = nc.dram_tensor("mask", [128, 128], BF16, kind="ExternalInput").ap()
    y_d = nc.dram_tensor("y", [BPC, NQC, 4, 9, QCH], F32,
                         kind="ExternalOutput").ap()

    with tile.TileContext(nc) as tc, ExitStack() as ctx:
        singles = ctx.enter_context(tc.tile_pool(name="singles", bufs=1))
        io_pool = ctx.enter_context(tc.tile_pool(name="io", bufs=2))
        e_pool = ctx.enter_context(tc.tile_pool(name="e", bufs=4))
        out_pool = ctx.enter_context(tc.tile_pool(name="out", bufs=2))
        s_pool = ctx.enter_context(tc.tile_pool(name="s", bufs=3, space="PSUM"))
        acc_pool = ctx.enter_context(
            tc.tile_pool(name="acc", bufs=2, space="PSUM"))

        mask_sb = singles.tile([128, 128], BF16)
        nc.sync.dma_start(out=mask_sb[:], in_=mk_d[:])

        # dummy activation with no deps: pulls the ~2.7us exp-table load
        # to kernel start, overlapping the input DMAs
        warm = singles.tile([1, 8], F32)
        nc.vector.memset(warm[:], 0.0)
        nc.scalar.activation(warm[:], warm[:],
                             mybir.ActivationFunctionType.Exp)

        # PE warmup burst: ~4us of full-mode matmuls un-throttles the HAM
        # clock gate before the real stream begins
        warm_w = singles.tile([128, 512], BF16)
        nc.vector.memset(warm_w[:], 0.0)
        warm_ps = acc_pool.tile([128, QCH], F32, tag="acc")
        for _ in range(10):
            nc.tensor.matmul(warm_ps[:], lhsT=warm_w[:, 0:128],
                             rhs=warm_w[:], start=True, stop=True)

        # persistent double-buffered q/k tiles: rows 0-2 hold the per-batch
        # projections (tiny DMAs), rows 3-127 zeroed once so the score
        # matmuls run a full-mode K=128 zero-padded contraction
        qk_sets = []
        for pi in range(2):
            q_sb = singles.tile([128, L], BF16, name=f"qsb{pi}")
            k_sb = singles.tile([128, L], BF16, name=f"ksb{pi}")
            nc.gpsimd.memset(q_sb[:], 0.0)
            nc.gpsimd.memset(k_sb[:], 0.0)
            qk_sets.append((q_sb, k_sb))

        # batches 0-1 run their q-chunks sequentially; batches 2-3 are
        # INTERLEAVED (big chunks paired with small) so the kernel tail
        # drains two independent dependency chains in parallel instead of
        # serializing one batch's exp->PV->copy chain
        units = [(b, qc) for b in (0, 1) for qc in range(NQC)]
        units += [(2, 3), (3, 3), (2, 2), (3, 2), (2, 1), (3, 1), (2, 0),
                  (3, 0)]
        vm_tiles = {}
        for b, qc in units:
            if b not in vm_tiles:
                q_sb, k_sb = qk_sets[b % 2]
                nc.sync.dma_start(out=q_sb[0:3, :], in_=q_d[b])
                nc.sync.dma_start(out=k_sb[0:3, :], in_=k_d[b])
                vm_sb = io_pool.tile([128, (NKT + 1) * 32], BF16, tag="vm")
                nc.sync.dma_start(out=vm_sb[:, 0:8 * 32],
                                  in_=vm_d[b][:, 0:8 * 32])
                nc.sync.dma_start(out=vm_sb[:, 8 * 32:(NKT + 1) * 32],
                                  in_=vm_d[b][:, 8 * 32:(NKT + 1) * 32])
                vm_tiles[b] = vm_sb
            q_sb, k_sb = qk_sets[b % 2]
            vm_sb = vm_tiles[b]
            if True:
                acc = acc_pool.tile([128, QCH], F32, tag="acc")
                strip_started = [False] * 4
                if b == 0:
                    # full-mode +0 accumulation: invisible to the output but
                    # HAM-visible filler that keeps the PE clock un-throttled
                    # through the ramp (the tiled PV matmuls don't count as
                    # PE activity for the HAM)
                    nc.tensor.matmul(acc[:], lhsT=warm_w[:, 0:128],
                                     rhs=warm_w[:], start=True, stop=False,
                                     skip_group_check=True)
                    strip_started = [True] * 4
                n_kt = 4 * qc + 4
                d0 = 4 * qc
                # groups of (kt, s-offset, width, acc-col-offset) sharing one
                # 2-bank PSUM tile and ONE gap-free exp each. Non-diagonal
                # tiles in pairs; the 4 diagonal staircase tiles pack
                # back-to-back as (j0,j1) -> 896 cols and (j2,j3) -> 384.
                groups = [[(2 * i, 0, QCH, 0), (2 * i + 1, QCH, QCH, 0)]
                          for i in range(2 * qc)]
                groups.append([(d0 + 0, 0, 512, 0),
                               (d0 + 1, 512, 384, 128)])
                groups.append([(d0 + 2, 0, 256, 256),
                               (d0 + 3, 256, 128, 384)])
                for gi, group in enumerate(groups):
                    is_diag = gi >= len(groups) - 2
                    use_dve = (not is_diag) and (gi % 2 == 0)
                    if b == 0 and gi > 0 and gi % 2 == 0:
                        nc.tensor.matmul(acc[:], lhsT=warm_w[:, 0:128],
                                         rhs=warm_w[:], start=False,
                                         stop=False, skip_group_check=True)
                    s = s_pool.tile([128, 2 * QCH], F32)
                    for kt, soff, w, co in group:
                        nc.tensor.matmul(
                            s[:, soff:soff + w],
                            lhsT=k_sb[:, kt * KTILE:(kt + 1) * KTILE],
                            rhs=q_sb[:, qc * QCH + co:(qc + 1) * QCH],
                            start=True, stop=True)
                    e = e_pool.tile([128, 2 * QCH], BF16)
                    hi = group[-1][1] + group[-1][2]
                    if use_dve:
                        nc.vector.tensor_scalar(
                            e[:, 0:hi].bitcast(I16), s[:, 0:hi],
                            FE_C1, FE_C2,
                            mybir.AluOpType.mult, mybir.AluOpType.add)
                    else:
                        nc.scalar.activation(
                            e[:, 0:hi], s[:, 0:hi],
                            mybir.ActivationFunctionType.Exp)
                    if is_diag:
                        for kt, soff, w, co in group:
                            nc.vector.tensor_mul(
                                e[:, soff:soff + 128], e[:, soff:soff + 128],
                                mask_sb[:])
                    if qc == 0 and gi == 0:
                        # zero-weight fills for the staircase's never-written
                        # column ranges so the acc bank is fully initialized
                        for ps in range(1, 4):
                            nc.tensor.matmul(
                                acc[32 * ps:32 * ps + 32, 0:128 * ps],
                                lhsT=vm_sb[:, NKT * 32:(NKT + 1) * 32],
                                rhs=e[:, 0:128 * ps],
                                start=not strip_started[ps], stop=False,
                                tile_position=(0, 32 * ps),
                                skip_group_check=True)
                            strip_started[ps] = True
                    for kt, soff, w, co in group:
                        ps = kt % 4
                        nc.tensor.matmul(
                            acc[32 * ps:32 * ps + 32, co:co + w],
                            lhsT=vm_sb[:, kt * 32:(kt + 1) * 32],
                            rhs=e[:, soff:soff + w],
                            start=not strip_started[ps],
                            stop=(kt >= d0),
                            tile_position=(0, 32 * ps),
                            skip_group_check=True)
                        strip_started[ps] = True

                out_sb = out_pool.tile([128, QCH], F32)
                nc.vector.tensor_copy(out_sb[:], acc[:])
                # per-chunk output DMAs on the (idle) gpsimd queue so they
                # never block the next batch's input DMAs on the sync queue;
                # the last batch's outputs use the lower-latency HWDGE sync
                # queue to shorten the tail
                out_eng = nc.sync if b == BPC - 1 else nc.gpsimd
                for ps in range(4):
                    out_eng.dma_start(
                        out=y_d[b, qc, ps],
                        in_=out_sb[32 * ps:32 * ps + 9, :])

    nc.compile()
    return nc


def _host_prep(inputs):
    """Fold the network's parameters into q/k projections and the VM matrix,
    and build per-core device inputs."""
    x = np.asarray(inputs["inputs"], dtype=np.float32)          # [B, L, 3]
    Wi = np.asarray(inputs["in_proj_w"], dtype=np.float64)      # [9, 3]
    bi = np.asarray(inputs["in_proj_b"], dtype=np.float64)      # [9]
    Wo = np.asarray(inputs["out_proj_w"], dtype=np.float64)     # [3, 3]
    bo = np.asarray(inputs["out_proj_b"], dtype=np.float64)     # [3]
    sigma = np.asarray(inputs["sigma"], dtype=np.float64)       # [2]
    f1_w = np.asarray(inputs["f1_w"], dtype=np.float64)
    f1_b = np.asarray(inputs["f1_b"], dtype=np.float64)
    f2_w = np.asarray(inputs["f2_w"], dtype=np.float64)
    f2_b = np.asarray(inputs["f2_b"], dtype=np.float64)
    g1_w = np.asarray(inputs["g1_w"], dtype=np.float64)
    g1_b = np.asarray(inputs["g1_b"], dtype=np.float64)
    g2_w = np.asarray(inputs["g2_w"], dtype=np.float64)
    g2_b = np.asarray(inputs["g2_b"], dtype=np.float64)
    m1 = float(np.asarray(inputs["m1_s"]))
    m2 = float(np.asarray(inputs["m2_s"]))

    scale = sigma + EPS
    dvec = np.array([1.0, 1.0 / scale[0], 1.0 / scale[1]])
    s3 = math.sqrt(3.0)

    Wq, Wk, Wv = Wi[0:3], Wi[3:6], Wi[6:9]
    bq, bk, bv = bi[0:3], bi[3:6], bi[6:9]
    Wq_eff = (Wq * dvec[None, :]) / s3
    bq_eff = bq / s3
    Wk_eff = Wk * dvec[None, :]
    bk_eff = bk
    Wv_eff = Wv * dvec[None, :]
    bv_eff = bv

    # affine collapse of the post-attention network: states are affine in
    # u = [1, a1, a2] (a = attention output channels 1, 2)
    e1 = np.array([1.0, 0.0, 0.0])

    def G(P):
        r1 = m1 * (g1_w @ P + g1_b[:, None] * e1[None, :])
        r2 = m2 * (g2_w @ P + g2_b[:, None] * e1[None, :])
        return np.vstack([np.zeros((1, 3)), r1, r2])

    P1 = np.eye(3)
    P2 = P1 + DT * G(P1)
    P3 = P2 + DT * G(P2)
    P4 = P3 + DT * G(P3)
    r7 = P4[1, :] + DT * m1 * (f1_w @ P4 + f1_b[:, None] * e1[None, :])[0]
    r8 = P4[2, :] + DT * m2 * (f2_w @ P4 + f2_b[:, None] * e1[None, :])[0]
    A = np.vstack([
        scale[0] * P2[1, :], scale[1] * P2[2, :],
        scale[0] * P3[1, :], scale[1] * P3[2, :],
        scale[0] * P4[1, :], scale[1] * P4[2, :],
        scale[0] * r7, scale[1] * r8,
    ])                                                  # [8, 3] in u-space
    U = np.zeros((3, 4))                                # u = U @ [ctx; 1]
    U[0, 3] = 1.0
    U[1, 0:3] = Wo[1, :]
    U[1, 3] = bo[1]
    U[2, 0:3] = Wo[2, :]
    U[2, 3] = bo[2]
    M = A @ U                                           # [8, 4]

    # vm: per-key row [ (V_ext @ M^T)[k], 1 ]  with V_ext = [V | 1]
    WvT_ext = np.zeros((4, 4))
    WvT_ext[0:3, 0:3] = Wv_eff.T
    WvT_ext[3, 0:3] = bv_eff
    WvT_ext[3, 3] = 1.0
    WVM = np.zeros((4, 9))
    WVM[:, 0:8] = WvT_ext @ M.T
    WVM[3, 8] = 1.0                     # softmax denominator column

    x_aug = np.concatenate([x, np.ones((B, L, 1), np.float32)], axis=-1)
    Wq_augT = np.concatenate([Wq_eff.T, bq_eff[None, :]],
                             axis=0).astype(np.float32)          # [4, 3]
    Wk_augT = np.concatenate([Wk_eff.T, bk_eff[None, :]],
                             axis=0).astype(np.float32)
    q_t = np.einsum("bld,dc->bcl", x_aug, Wq_augT)               # [B, 3, L]
    k_t = np.einsum("bld,dc->bcl", x_aug, Wk_augT)
    vm = x_aug @ WVM.astype(np.float32)                          # [B, L, 9]

    q_host = q_t.astype(BF16_NP)                                 # [B, 3, L]
    k_host = k_t.astype(BF16_NP)

    # vm per key tile, padded to 32 cols (9 real + zeros) + one zero slot
    vm_pad = np.zeros((B, NKT + 1, KTILE, 32), dtype=BF16_NP)
    vm_pad[:, 0:NKT, :, 0:9] = vm.reshape(B, NKT, KTILE, 9).astype(BF16_NP)
    vm_dev = np.ascontiguousarray(
        vm_pad.transpose(0, 2, 1, 3).reshape(B, KTILE, (NKT + 1) * 32))

    mask = (np.arange(128)[None, :] >=
            np.arange(128)[:, None]).astype(BF16_NP)
    in_maps = []
    for c in range(NCORES):
        sl = slice(c * BPC, (c + 1) * BPC)
        in_maps.append({
            "q": np.ascontiguousarray(q_host[sl]),
            "k": np.ascontiguousarray(k_host[sl]),
            "vm": np.ascontiguousarray(vm_dev[sl]),
            "mask": mask,
        })
    return in_maps


def _unshard(y):
    """[B, NQC, 4, 9, QCH] strip partials -> [B, L, 8] output."""
    # zero the never-written acc regions of qc=0 (strip s valid from col 128s)
    for s in range(1, 4):
        y[:, 0, s, :, 0:128 * s] = 0.0
    acc = y.sum(axis=2)                               # [B, NQC, 9, QCH]
    nb = y.shape[0]
    acc = acc.transpose(0, 2, 1, 3).reshape(nb, 9, L)
    num = acc[:, 0:8, :]
    den = acc[:, 8:9, :]
    out = (num / den).transpose(0, 2, 1)              # [B, L, 8]
    return np.ascontiguousarray(out.astype(np.float32))


def kernel(**inputs) -> np.ndarray:
    global _built, LAST_EXEC_TIME_NS
    if _built is None:
        _built = _build()
    nc = _built

    in_maps = _host_prep(inputs)

    trace = os.environ.get("BASS_KERNEL_TRACE", "") == "1"
    res = run_bass_kernel_spmd(nc, in_maps, list(range(NCORES)), trace=trace)
    if trace:
        LAST_EXEC_TIME_NS = res.exec_time_ns

    y = np.concatenate([res.results[c]["y"] for c in range(NCORES)],
                       axis=0)                        # [B, NQC, 4, 9, QCH]
    return _unshard(y)
